# revision 32
# baseline (speedup 1.0000x reference)
"""BiMamba (bimamba_type='v2') Trainium2 Bass kernel.

Data-parallel over the fused B*N=828 (padded to 832) sequence axis across 8
NeuronCores (104 sequences/core). Per-core device program:
  - channels d (d_inner=256) -> 2 partition tiles of 128
  - scan-phase tensors laid out [p=d-tile, (branch, n_state, batch, time)];
    the selective scan runs as two DVE tensor_tensor_scan per (d-tile,
    chunk): branch 0 forward, branch 1 via time-reversed APs (all branch-1
    tensors stay forward-oriented; only the scan walks t backwards).
    dA is zeroed at the per-(n,seq) segment start (t=0 fwd / t=T-1 bwd).
  - depthwise causal conv runs on the PE as 4 shifted diagonal matmuls
    accumulating in PSUM (branch 1 uses the anti-causal shifts).
  - dBu = du (x) B_rep runs on the GpSimd (Pool) engine, produced half a
    chunk ahead of its scan so the DVE never waits on it.
  - activation tables restricted to {natural_log_exp_and_others,
    silu_and_others}: softplus = ln(1+exp), rstd = exp(-0.5*ln(var+eps)),
    so exp/ln share one table -> 2 table loads per chunk.
  - LN folded: hln = u*(g*rstd)_bcast - (g*mean*rstd - b)_bcast, with both
    broadcasts built by PE matmuls (g/b as 1-row weights); DVE does 2
    passes + 3 tiny [1,CBT] ops per LN.
  - all matmuls on bf16 operands (fp32 PSUM accumulate); PSUM->SBUF
    copies/casts/activations on the Act engine.
  - chunk pipeline: front(c+1) | back_p1(c) | mid(c+1) | back_p2(c), with
    LN2(c)+residual deferred to the next iteration, so the Act/PE/Pool
    producer chains hide under the DVE scan work.
"""

import numpy as np
import ml_dtypes

import concourse.bass as bass
import concourse.tile as tile
from concourse import bacc, mybir
from concourse.bass_utils import run_bass_kernel_spmd

F32 = mybir.dt.float32
BF16 = mybir.dt.bfloat16
AF = mybir.ActivationFunctionType
ALU = mybir.AluOpType

B, T, N, C = 4, 24, 207, 128
DI = 256
DS = 16
RK = 8
EPS = 1e-5
NCORES = 8
BSEQ = 832                   # padded B*N
BC = BSEQ // NCORES          # 104 sequences per core
NCHUNK = 8
CB = BC // NCHUNK            # 13 seqs per chunk
CBT = CB * T                 # 312 tokens per chunk

BF = ml_dtypes.bfloat16


def _pbcast(ap, parts=128):
    """DRAM-source AP replicating data across `parts` partitions."""
    a = [[0, parts]] + [list(x) for x in ap.ap]
    return bass.AP(tensor=ap.tensor, offset=ap.offset, ap=a)


def _rev_t(ap):
    """Reverse the last free dim of an AP."""
    a = [list(x) for x in ap.ap]
    st, ct = a[-1]
    off = ap.offset + st * (ct - 1)
    a[-1] = [-st, ct]
    return bass.AP(tensor=ap.tensor, offset=off, ap=a)


def _zstride(ap, dim, count):
    """Insert a 0-stride free dim at position `dim`."""
    a = [list(x) for x in ap.ap]
    a.insert(1 + dim, [0, count])
    return bass.AP(tensor=ap.tensor, offset=ap.offset, ap=a)


def build_program(a_pow):
    _gat = bacc.get_activation_tables

    def _patched(arch):
        t = _gat(arch)
        keep = ("natural_log_exp_and_others", "silu_and_others")
        return {k: (v if k in keep else set()) for k, v in t.items()}

    bacc.get_activation_tables = _patched
    try:
        return _build_program(a_pow)
    finally:
        bacc.get_activation_tables = _gat


def _build_program(a_pow):
    nc = bacc.Bacc("TRN2", target_bir_lowering=False, debug=False,
                   enable_asserts=False, num_devices=NCORES)

    def din(name, shape, dt=F32):
        return nc.dram_tensor(name, shape, dt, kind="ExternalInput").ap()

    xin = din("xin", [C, BC, T])
    w_in = din("w_in", [C, 4 * C], BF16)
    convd = din("convd", [128, 2, 2, 4, 128], BF16)   # diag conv weights
    convb = din("convb", [128, 2, 2, 1])
    xw = din("xw", [128, 2, 2, 40], BF16)
    dtw = din("dtw", [RK, 2, DI], BF16)
    dtb = din("dtb", [128, 2, 2, 1])
    dpc = din("dpc", [128, 2, 2, 1])
    wout = din("wout", [128, 2, C], BF16)
    lnw = din("lnw", [1, 4, C])          # rows: ln1g ln1b ln2g ln2b
    out = nc.dram_tensor("out", [C, BC, T], F32, kind="ExternalOutput").ap()

    with tile.TileContext(nc) as tc, \
         tc.tile_pool(name="weights", bufs=1) as wp, \
         tc.tile_pool(name="small", bufs=2) as sp, \
         tc.tile_pool(name="stats", bufs=2) as stp, \
         tc.tile_pool(name="big", bufs=2) as bp, \
         tc.tile_pool(name="bcrep", bufs=1) as bcp, \
         tc.tile_pool(name="dram", bufs=2, space="DRAM") as drp, \
         tc.tile_pool(name="psA", bufs=2, space="PSUM") as psA, \
         tc.tile_pool(name="psB", bufs=2, space="PSUM") as psB, \
         tc.tile_pool(name="psC", bufs=2, space="PSUM") as psC:

        def load_w(name, ap_src, shape, dt=F32, eng=nc.sync):
            t = wp.tile(shape, dt, tag=name, name=name)
            eng.dma_start(t[:], ap_src)
            return t

        w_in_sb = load_w("w_in", w_in, [C, 4 * C], BF16)
        convd_sb = load_w("convd", convd, [128, 2, 2, 4, 128], BF16,
                          eng=nc.scalar)
        convb_sb = load_w("convb", convb, [128, 2, 2, 1], eng=nc.gpsimd)
        xw_sb = load_w("xw", xw, [128, 2, 2, 40], BF16, eng=nc.scalar)
        dtw_sb = load_w("dtw", dtw, [RK, 2, DI], BF16, eng=nc.scalar)
        dtb_sb = load_w("dtb", dtb, [128, 2, 2, 1], eng=nc.gpsimd)
        dpc_sb = load_w("dpc", dpc, [128, 2, 2, 1], eng=nc.gpsimd)
        wout_sb = load_w("wout", wout, [128, 2, C], BF16, eng=nc.gpsimd)
        lnw_sb = load_w("lnw", lnw, [1, 4, C], F32, eng=nc.scalar)
        lnw_bf = wp.tile([1, 4, C], BF16, tag="lnw_bf")
        nc.scalar.copy(lnw_bf[:], lnw_sb[:])
        g_row = [lnw_bf[:, 0, :], lnw_bf[:, 2, :]]
        b_row = [lnw_bf[:, 1, :], lnw_bf[:, 3, :]]
        ones_bf = wp.tile([C, 1], BF16, tag="ones_bf")
        nc.vector.memset(ones_bf[:], 1.0)
        eps_sb = wp.tile([C, 1], F32, tag="eps")
        nc.vector.memset(eps_sb[:], EPS)
        neg1_row = wp.tile([1, CBT], BF16, tag="neg1_row")
        nc.vector.memset(neg1_row[:], -1.0)

        def layernorm(ln_i, src_f32, src_bf, sq_bf, dst):
            """dst = (src - mean)/std * g + b over the partition dim.
            Folded: dst = src*(g*rstd)_r - (g*mean*rstd - b)_r."""
            ps_s = psC.tile([1, CBT], F32, tag="ps_stat", name="ps_s")
            ps_q = psC.tile([1, CBT], F32, tag="ps_stat", name="ps_q")
            nc.tensor.matmul(ps_s[:], ones_bf[:], src_bf, start=True, stop=True)
            nc.tensor.matmul(ps_q[:], ones_bf[:], sq_bf, start=True, stop=True)
            s_sb = stp.tile([1, CBT], F32, tag="s_sb")
            nc.scalar.copy(s_sb[:], ps_s[:])
            m2 = stp.tile([1, CBT], F32, tag="m2")
            nc.vector.tensor_mul(m2[:], s_sb[:], s_sb[:])
            varI = stp.tile([1, CBT], F32, tag="varI")
            # varI = C*ps_q - ps_s^2 = C^2 * var
            nc.vector.scalar_tensor_tensor(varI[:], ps_q[:], float(C), m2[:],
                                           ALU.mult, ALU.subtract)
            # rstd = exp(-0.5*ln(varI/C^2 + eps))
            nc.scalar.activation(varI[:], varI[:], AF.Ln,
                                 scale=1.0 / (C * C), bias=eps_sb[0:1, 0:1])
            nc.scalar.activation(varI[:], varI[:], AF.Exp, scale=-0.5)
            rstd_bf = stp.tile([1, CBT], BF16, tag="rstd_bf")
            nc.scalar.copy(rstd_bf[:], varI[:])
            # mr = mean*rstd = (ps_s/C)*rstd
            mr_bf = stp.tile([1, CBT], BF16, tag="mr_bf")
            nc.vector.scalar_tensor_tensor(mr_bf[:], s_sb[:], 1.0 / C,
                                           varI[:], ALU.mult, ALU.mult)
            grstd_r = psC.tile([C, CBT], F32, tag="ps_stat", name="grstd_r")
            nc.tensor.matmul(grstd_r[:], g_row[ln_i], rstd_bf[:],
                             start=True, stop=True)
            mgb_r = psC.tile([C, CBT], F32, tag="ps_stat", name="mgb_r")
            nc.tensor.matmul(mgb_r[:], b_row[ln_i], neg1_row[:],
                             start=True, stop=False)
            nc.tensor.matmul(mgb_r[:], g_row[ln_i], mr_bf[:],
                             start=False, stop=True)
            tln = sp.tile([C, CBT], F32, tag="ln_tmp")
            nc.vector.tensor_mul(tln[:], src_f32, grstd_r[:])
            nc.vector.tensor_sub(dst, tln[:], mgb_r[:])

        def front_head(ch):
            """u DMA + bf16 casts + LN1 stat matmuls (Act/PE only)."""
            b0 = ch * CB
            u = sp.tile([C, CB, T], F32, tag="u", bufs=3)
            nc.sync.dma_start(u[:], xin[:, b0:b0 + CB, :])
            uf = u[:].rearrange("p b t -> p (b t)")
            u_bf = sp.tile([C, CBT], BF16, tag="u_bf")
            nc.scalar.copy(u_bf[:], uf)
            sq_bf = sp.tile([C, CBT], BF16, tag="sq_bf")
            nc.scalar.square(sq_bf[:], uf)
            ps_s = psC.tile([1, CBT], F32, tag="ps_stat", name="ps_s")
            ps_q = psC.tile([1, CBT], F32, tag="ps_stat", name="ps_q")
            nc.tensor.matmul(ps_s[:], ones_bf[:], u_bf[:],
                             start=True, stop=True)
            nc.tensor.matmul(ps_q[:], ones_bf[:], sq_bf[:],
                             start=True, stop=True)
            s_sb = stp.tile([1, CBT], F32, tag="s_sb")
            nc.scalar.copy(s_sb[:], ps_s[:])
            return dict(ch=ch, uf=uf, ps_q=ps_q, s_sb=s_sb)

        def front_rest(hd):
            """LN1 stats (DVE) + apply + in_proj + conv(PE) + xproj +
            B/C broadcast + dtproj + dA(ti=0)."""
            ch, uf = hd["ch"], hd["uf"]
            ps_q, s_sb = hd["ps_q"], hd["s_sb"]
            m2 = stp.tile([1, CBT], F32, tag="m2")
            nc.vector.tensor_mul(m2[:], s_sb[:], s_sb[:])
            varI = stp.tile([1, CBT], F32, tag="varI")
            nc.vector.scalar_tensor_tensor(varI[:], ps_q[:], float(C), m2[:],
                                           ALU.mult, ALU.subtract)
            nc.scalar.activation(varI[:], varI[:], AF.Ln,
                                 scale=1.0 / (C * C), bias=eps_sb[0:1, 0:1])
            nc.scalar.activation(varI[:], varI[:], AF.Exp, scale=-0.5)
            rstd_bf = stp.tile([1, CBT], BF16, tag="rstd_bf")
            nc.scalar.copy(rstd_bf[:], varI[:])
            mr_bf = stp.tile([1, CBT], BF16, tag="mr_bf")
            nc.vector.scalar_tensor_tensor(mr_bf[:], s_sb[:], 1.0 / C,
                                           varI[:], ALU.mult, ALU.mult)
            grstd_r = psC.tile([C, CBT], F32, tag="ps_stat", name="grstd_r")
            nc.tensor.matmul(grstd_r[:], g_row[0], rstd_bf[:],
                             start=True, stop=True)
            mgb_r = psC.tile([C, CBT], F32, tag="ps_stat", name="mgb_r")
            nc.tensor.matmul(mgb_r[:], b_row[0], neg1_row[:],
                             start=True, stop=False)
            nc.tensor.matmul(mgb_r[:], g_row[0], mr_bf[:],
                             start=False, stop=True)
            hln = sp.tile([C, CBT], BF16, tag="hln")
            tln = sp.tile([C, CBT], F32, tag="ln_tmp")
            nc.vector.tensor_mul(tln[:], uf, grstd_r[:])
            nc.vector.tensor_sub(hln[:], tln[:], mgb_r[:])

            # in_proj (m-tiles: xx0 xx1 z0 z1)
            xx = [sp.tile([128, CB, T], BF16, tag=f"xx{ti}", name=f"xx{ti}")
                  for ti in range(2)]
            sz = [sp.tile([128, CB, T], BF16, tag=f"sz{ti}", name=f"sz{ti}")
                  for ti in range(2)]
            for mt in range(4):
                ps_xz = psA.tile([128, CBT], F32, tag="ps_xz")
                nc.tensor.matmul(ps_xz[:], w_in_sb[:, mt * 128:(mt + 1) * 128],
                                 hln[:], start=True, stop=True)
                dst = xx[mt][:] if mt < 2 else sz[mt - 2][:]
                pv = ps_xz[:].rearrange("p (b t) -> p b t", t=T)
                if mt < 2:
                    nc.scalar.copy(dst, pv)
                else:
                    nc.scalar.activation(dst, pv, AF.Silu)

            # depthwise conv on PE: shifted diagonal matmuls
            # xc2 combined [p, br, ti, b, t]
            xc2 = sp.tile([128, 2, 2, CB, T], BF16, tag="xc", name="xc2")
            for ti in range(2):
                xxv = xx[ti][:]
                for br in range(2):
                    ps_cv = psA.tile([128, CB, T], F32, tag="ps_xz",
                                     name=f"ps_cv{ti}{br}")
                    nc.tensor.matmul(ps_cv[:], convd_sb[:, br, ti, 3, :],
                                     xxv, start=True, stop=False)
                    for k in range(3):
                        s = 3 - k
                        if br == 0:
                            dst, src = ps_cv[:, :, s:], xxv[:, :, :T - s]
                        else:
                            dst, src = ps_cv[:, :, :T - s], xxv[:, :, s:]
                        nc.tensor.matmul(dst, convd_sb[:, br, ti, k, :], src,
                                         start=False, stop=(k == 2))
                    nc.scalar.activation(xc2[:, br, ti, :, :], ps_cv[:],
                                         AF.Silu, bias=convb_sb[:, br, ti, 0:1])

            # xproj -> x_dbl [40, CBT] per branch; dtproj chained per
            # branch right after its dtraw copy (shortest path to dt2);
            # each branch's B/C staging DMA issues as soon as its bc_bf
            # copy lands so the broadcast round trip starts earliest
            dtraw, bc_bf = [None, None], [None, None]
            dt2 = sp.tile([128, 2, 2, CB, T], BF16, tag="dt", name="dt2")
            b1d = drp.tile([2, DS, CB, T], BF16, tag="b1d")
            c1d = drp.tile([2, DS, CB, T], BF16, tag="c1d")
            for br in range(2):
                ps_xd = psA.tile([40, CBT], F32, tag="ps_m",
                                 name=f"ps_xd{br}")
                for ti in range(2):
                    nc.tensor.matmul(ps_xd[:], xw_sb[:, br, ti, :],
                                     xc2[:, br, ti, :, :].rearrange(
                                         "p b t -> p (b t)"),
                                     start=(ti == 0), stop=(ti == 1))
                dtraw[br] = stp.tile([RK, CBT], BF16, tag=f"dtraw{br}",
                                     name=f"dtraw{br}")
                nc.scalar.copy(dtraw[br][:], ps_xd[32:40, :])
                bc_bf[br] = stp.tile([32, CBT], BF16, tag=f"bcbf{br}",
                                     name=f"bcbf{br}")
                nc.scalar.copy(bc_bf[br][:], ps_xd[0:32, :])
                nc.sync.dma_start(b1d[br, :, :, :],
                                  bc_bf[br][0:DS, :].rearrange(
                                      "p (b t) -> p b t", t=T))
                nc.sync.dma_start(c1d[br, :, :, :],
                                  bc_bf[br][DS:32, :].rearrange(
                                      "p (b t) -> p b t", t=T))
                for ti in range(2):
                    ps_dt = psA.tile([128, CBT], F32, tag="ps_m",
                                     name=f"ps_dt{br}{ti}")
                    nc.tensor.matmul(ps_dt[:],
                                     dtw_sb[:, br, ti * 128:(ti + 1) * 128],
                                     dtraw[br][:], start=True, stop=True)
                    slab = dt2[:, br, ti, :, :]
                    nc.scalar.activation(
                        slab, ps_dt[:].rearrange("p (b t) -> p b t", t=T),
                        AF.Exp, bias=dtb_sb[:, br, ti, 0:1])
                    nc.scalar.activation(slab, slab, AF.Ln, bias=1.0)

            brep = bcp.tile([128, 2 * DS * CBT], BF16, tag="brep")
            nc.sync.dma_start(
                brep[:], _pbcast(b1d[:].rearrange("a n b t -> (a n b t)")))
            crep = bcp.tile([128, 2 * DS * CBT], BF16, tag="crep")
            nc.sync.dma_start(
                crep[:], _pbcast(c1d[:].rearrange("a n b t -> (a n b t)")))


            st = dict(ch=ch, uf=uf, xc2=xc2[:], sz=sz, dt2=dt2[:],
                      brep=brep, crep=crep, dA=None, dBu=None)
            prep_act(st)
            return st

        def prep_act(st):
            """dA (Act, both d-tiles per act) + segment resets (Pool).
            dA layout [p, br, ti, n, b, t]; the act dst slab for fixed n
            iterates (br, ti, b, t) matching dt2's element order."""
            dt2 = st["dt2"]
            dA = bp.tile([128, 2, 2, DS, CB, T], BF16, tag="dA",
                         bufs=1)
            for n in range(DS):
                nc.scalar.activation(dA[:, :, :, n, :, :], dt2[:],
                                     AF.Exp, scale=float(a_pow[n]))
            nc.gpsimd.memset(dA[:, 0, :, :, :, 0:1], 0.0)
            nc.gpsimd.memset(dA[:, 1, :, :, :, T - 1:T], 0.0)
            st["dA"] = dA

        def prep_mul(st):
            """du2 + dBu (DVE), one combined op each; emitted after back(c)
            so dtproj(c+1) has a full chunk of slack."""
            dt2, xc2 = st["dt2"], st["xc2"]
            brep6 = _zstride(st["brep"][:].rearrange(
                "p (a n b t) -> p a n b t", a=2, n=DS, b=CB), 1, 2)
            du2 = bp.tile([128, 2, 2, CB, T], BF16, tag="du2")
            nc.vector.tensor_mul(du2[:], dt2[:], xc2[:])
            dBu = bp.tile([128, 2, 2, DS, CB, T], BF16, tag="dBu",
                          bufs=1)
            nc.vector.tensor_mul(dBu[:], _zstride(du2[:], 2, DS), brep6)
            st["dBu"] = dBu

        def back(st):
            """Per-branch merged scan over (ti,n,b,t) + hc + n-reduce +
            gating + out_proj accumulate."""
            xc2, sz = st["xc2"], st["sz"]
            dA, dBu = st["dA"], st["dBu"]
            crep6 = _zstride(st["crep"][:].rearrange(
                "p (a n b t) -> p a n b t", a=2, n=DS, b=CB), 1, 2)
            st["ps_o"] = psB.tile([128, CBT], F32, tag="ps_o", name="ps_o")
            ps_o = st["ps_o"]
            flat = "p ti n b t -> p (ti n b t)"

            # scan in place: h overwrites dA (per-element read precedes
            # the same element's write in the DVE pipeline)
            h = dA
            nc.vector.tensor_tensor_scan(
                h[:, 0].rearrange(flat), dA[:, 0].rearrange(flat),
                dBu[:, 0].rearrange(flat), 0.0, ALU.mult, ALU.add)
            nc.vector.tensor_tensor_scan(
                _rev_t(h[:, 1].rearrange(flat)),
                _rev_t(dA[:, 1].rearrange(flat)),
                _rev_t(dBu[:, 1].rearrange(flat)),
                0.0, ALU.mult, ALU.add)
            nc.vector.tensor_mul(h[:], h[:], crep6)
            for w in (8, 4, 2, 1):
                nc.vector.tensor_add(h[:, :, :, 0:w, :, :],
                                     h[:, :, :, 0:w, :, :],
                                     h[:, :, :, w:2 * w, :, :])
            for br in range(2):
                for ti in range(2):
                    y = sp.tile([128, CB, T], BF16, tag=f"y{br}{ti}",
                                name=f"y{br}{ti}")
                    nc.vector.scalar_tensor_tensor(
                        y[:], xc2[:, br, ti, :, :], dpc_sb[:, br, ti, 0:1],
                        h[:, br, ti, 0, :, :], ALU.mult, ALU.add)
                    st[f"y{br}{ti}"] = y
            for ti in range(2):
                ypre = sp.tile([128, CB, T], BF16, tag=f"ypre{ti}",
                               name=f"ypre{ti}")
                nc.vector.tensor_add(ypre[:], st[f"y0{ti}"][:],
                                     st[f"y1{ti}"][:])
                nc.vector.tensor_mul(ypre[:], ypre[:], sz[ti][:])
                nc.tensor.matmul(ps_o[:], wout_sb[:, ti, :],
                                 ypre[:].rearrange("p b t -> p (b t)"),
                                 start=(ti == 0), stop=(ti == 1))

        def tail(st):
            """LN2 + residual + store (runs one iteration later)."""
            ch, uf, ps_o = st["ch"], st["uf"], st["ps_o"]
            b0 = ch * CB
            o_sb = sp.tile([C, CBT], F32, tag="o_sb")
            nc.scalar.copy(o_sb[:], ps_o[:])
            o_bf = sp.tile([C, CBT], BF16, tag="o_bf")
            nc.scalar.copy(o_bf[:], ps_o[:])
            sq2_bf = sp.tile([C, CBT], BF16, tag="sq2_bf")
            nc.scalar.square(sq2_bf[:], ps_o[:])
            o_ln = sp.tile([C, CBT], F32, tag="o_ln")
            layernorm(1, o_sb[:], o_bf[:], sq2_bf[:], o_ln[:])
            nc.vector.tensor_add(o_ln[:], o_ln[:], uf)
            nc.sync.dma_start(out[:, b0:b0 + CB, :],
                              o_ln[:].rearrange("p (b t) -> p b t", t=T))

        # pipeline per iteration (steady state):
        #   tail(c-1) | front_head(c+1) | p1(c) | front_rest(c+1) | p2(c)
        #   | du2/dBu/dA(c+1)
        st = front_rest(front_head(0))
        prep_mul(st)
        done = None
        for ch in range(NCHUNK):
            nxt = front_rest(front_head(ch + 1)) \
                if ch + 1 < NCHUNK else None
            back(st)
            if done is not None:
                tail(done)
            if nxt is not None:
                prep_mul(nxt)
            done = st
            st = nxt
        tail(done)

    nc.finalize()
    return nc


def _prep(inputs):
    f = lambda k: np.ascontiguousarray(np.asarray(inputs[k], np.float32))
    bf = lambda a: np.ascontiguousarray(np.asarray(a, BF))
    x = f("x")
    u_all = x.transpose(0, 2, 1, 3).reshape(B * N, T, C)
    u_pad = np.zeros((BSEQ, T, C), np.float32)
    u_pad[:B * N] = u_all
    xin = [np.ascontiguousarray(u_pad[i * BC:(i + 1) * BC].transpose(2, 0, 1))
           for i in range(NCORES)]

    A = -np.exp(f("A_log"))
    Ab = -np.exp(f("A_b_log"))
    assert np.allclose(A, A[0:1], rtol=1e-5), "A must be d-independent"
    assert np.allclose(Ab, A, rtol=1e-5), "A_b must equal A"
    a_pow = [float(v) for v in A[0]]

    cw = np.stack([f("conv_w")[:, 0, :], f("conv_w_b")[:, 0, :]])   # [2,256,4]
    cb = np.stack([f("conv_b"), f("conv_b_b")])[..., None]          # [2,256,1]
    convd = np.zeros((128, 2, 2, 4, 128), np.float32)
    rng = np.arange(128)
    for br in range(2):
        for ti in range(2):
            for k in range(4):
                convd[rng, br, ti, k, rng] = cw[br, ti * 128:(ti + 1) * 128, k]
    xw_ro = np.concatenate([f("xproj_w")[RK:], f("xproj_w")[:RK]])
    xw_ro_b = np.concatenate([f("xproj_w_b")[RK:], f("xproj_w_b")[:RK]])
    xwm = np.stack([xw_ro, xw_ro_b]).transpose(0, 2, 1)
    dtwm = np.stack([f("dtproj_w"), f("dtproj_w_b")]).transpose(0, 2, 1)
    dtbm = np.stack([f("dtproj_b"), f("dtproj_b_b")])[..., None]
    shared = {
        "w_in": bf(f("in_proj_w").T),
        "convd": bf(convd),
        "convb": np.ascontiguousarray(
            cb.reshape(2, 2, 128, 1).transpose(2, 0, 1, 3)),
        "xw": bf(xwm.reshape(2, 2, 128, 40).transpose(2, 0, 1, 3)),
        "dtw": bf(dtwm.transpose(1, 0, 2)),                         # [8,2,256]
        "dtb": np.ascontiguousarray(
            dtbm.reshape(2, 2, 128, 1).transpose(2, 0, 1, 3)),
        "dpc": np.ascontiguousarray(
            np.stack([f("Dp"), f("Dp_b")])[..., None]
            .reshape(2, 2, 128, 1).transpose(2, 0, 1, 3)),
        "wout": bf(f("out_proj_w").T.reshape(2, 128, 128).transpose(1, 0, 2)),
        "lnw": np.ascontiguousarray(
            np.stack([f("ln1_g"), f("ln1_b"), f("ln2_g"), f("ln2_b")])
            .reshape(1, 4, C)),
    }
    return xin, shared, a_pow


def _unshard(core_outs):
    y = np.stack(core_outs)                       # [8, C, BC, T]
    y = y.transpose(0, 2, 3, 1).reshape(BSEQ, T, C)[:B * N]
    return np.ascontiguousarray(
        y.reshape(B, N, T, C).transpose(0, 2, 1, 3))


_CACHE = {}


def kernel(_trace=False, **inputs):
    xin, shared, a_pow = _prep(inputs)
    if "prog" not in _CACHE:
        _CACHE["prog"] = build_program(a_pow)
    nc = _CACHE["prog"]
    in_maps = [dict(shared, xin=xin[i]) for i in range(NCORES)]
    res = run_bass_kernel_spmd(nc, in_maps, core_ids=list(range(NCORES)),
                               trace=_trace)
    out = _unshard([r["out"] for r in res.results])
    if _trace:
        kernel.last_results = res
    return out


# revision 33
# speedup vs baseline: 1.1840x; 1.1840x over previous
"""BiMamba (bimamba_type='v2') Trainium2 Bass kernel.

Data-parallel over the fused B*N=828 (padded to 832) sequence axis across 8
NeuronCores (104 sequences/core). Per-core device program:
  - channels d (d_inner=256) -> 2 partition tiles of 128
  - scan-phase tensors laid out [p=d-tile, (branch, n_state, batch, time)];
    the selective scan runs as two DVE tensor_tensor_scan per (d-tile,
    chunk): branch 0 forward, branch 1 via time-reversed APs (all branch-1
    tensors stay forward-oriented; only the scan walks t backwards).
    dA is zeroed at the per-(n,seq) segment start (t=0 fwd / t=T-1 bwd).
  - depthwise causal conv runs on the PE as 4 shifted diagonal matmuls
    accumulating in PSUM (branch 1 uses the anti-causal shifts).
  - dBu = du (x) B_rep runs on the GpSimd (Pool) engine, produced half a
    chunk ahead of its scan so the DVE never waits on it.
  - activation tables restricted to {natural_log_exp_and_others,
    silu_and_others}: softplus = ln(1+exp), rstd = exp(-0.5*ln(var+eps)),
    so exp/ln share one table -> 2 table loads per chunk.
  - LN folded: hln = u*(g*rstd)_bcast - (g*mean*rstd - b)_bcast, with both
    broadcasts built by PE matmuls (g/b as 1-row weights); DVE does 2
    passes + 3 tiny [1,CBT] ops per LN.
  - all matmuls on bf16 operands (fp32 PSUM accumulate); PSUM->SBUF
    copies/casts/activations on the Act engine.
  - chunk pipeline: front(c+1) | back_p1(c) | mid(c+1) | back_p2(c), with
    LN2(c)+residual deferred to the next iteration, so the Act/PE/Pool
    producer chains hide under the DVE scan work.
"""

import numpy as np
import ml_dtypes

import concourse.bass as bass
import concourse.tile as tile
from concourse import bacc, mybir
from concourse.bass_utils import run_bass_kernel_spmd

F32 = mybir.dt.float32
BF16 = mybir.dt.bfloat16
AF = mybir.ActivationFunctionType
ALU = mybir.AluOpType

B, T, N, C = 4, 24, 207, 128
DI = 256
DS = 16
RK = 8
EPS = 1e-5
NCORES = 8
BSEQ = 832                   # padded B*N
BC = BSEQ // NCORES          # 104 sequences per core
NCHUNK = 8
CB = BC // NCHUNK            # 13 seqs per chunk
CBT = CB * T                 # 312 tokens per chunk

BF = ml_dtypes.bfloat16


def _pbcast(ap, parts=128):
    """DRAM-source AP replicating data across `parts` partitions."""
    a = [[0, parts]] + [list(x) for x in ap.ap]
    return bass.AP(tensor=ap.tensor, offset=ap.offset, ap=a)


def _rev_t(ap):
    """Reverse the last free dim of an AP."""
    a = [list(x) for x in ap.ap]
    st, ct = a[-1]
    off = ap.offset + st * (ct - 1)
    a[-1] = [-st, ct]
    return bass.AP(tensor=ap.tensor, offset=off, ap=a)


def _zstride(ap, dim, count):
    """Insert a 0-stride free dim at position `dim`."""
    a = [list(x) for x in ap.ap]
    a.insert(1 + dim, [0, count])
    return bass.AP(tensor=ap.tensor, offset=ap.offset, ap=a)


def build_program(a_pow):
    _gat = bacc.get_activation_tables

    def _patched(arch):
        t = _gat(arch)
        keep = ("natural_log_exp_and_others", "silu_and_others")
        return {k: (v if k in keep else set()) for k, v in t.items()}

    bacc.get_activation_tables = _patched
    try:
        return _build_program(a_pow)
    finally:
        bacc.get_activation_tables = _gat


def _build_program(a_pow):
    nc = bacc.Bacc("TRN2", target_bir_lowering=False, debug=False,
                   enable_asserts=False, num_devices=NCORES)

    def din(name, shape, dt=F32):
        return nc.dram_tensor(name, shape, dt, kind="ExternalInput").ap()

    xin = din("xin", [C, BC, T])
    w_in = din("w_in", [C, 4 * C], BF16)
    convd = din("convd", [128, 2, 2, 4, 128], BF16)   # diag conv weights
    convb = din("convb", [128, 2, 2, 1])
    xw = din("xw", [128, 2, 2, 40], BF16)
    dtw = din("dtw", [RK, 2, DI], BF16)
    dtb = din("dtb", [128, 2, 2, 1])
    dpc = din("dpc", [128, 2, 2, 1])
    wout = din("wout", [128, 2, C], BF16)
    lnw = din("lnw", [1, 4, C])          # rows: ln1g ln1b ln2g ln2b
    out = nc.dram_tensor("out", [C, BC, T], F32, kind="ExternalOutput").ap()

    with tile.TileContext(nc) as tc, \
         tc.tile_pool(name="weights", bufs=1) as wp, \
         tc.tile_pool(name="small", bufs=2) as sp, \
         tc.tile_pool(name="stats", bufs=2) as stp, \
         tc.tile_pool(name="big", bufs=2) as bp, \
         tc.tile_pool(name="bcrep", bufs=1) as bcp, \
         tc.tile_pool(name="dram", bufs=2, space="DRAM") as drp, \
         tc.tile_pool(name="psA", bufs=2, space="PSUM") as psA, \
         tc.tile_pool(name="psB", bufs=2, space="PSUM") as psB, \
         tc.tile_pool(name="psC", bufs=2, space="PSUM") as psC:

        def load_w(name, ap_src, shape, dt=F32, eng=nc.sync):
            t = wp.tile(shape, dt, tag=name, name=name)
            eng.dma_start(t[:], ap_src)
            return t

        w_in_sb = load_w("w_in", w_in, [C, 4 * C], BF16)
        convd_sb = load_w("convd", convd, [128, 2, 2, 4, 128], BF16,
                          eng=nc.scalar)
        convb_sb = load_w("convb", convb, [128, 2, 2, 1], eng=nc.gpsimd)
        xw_sb = load_w("xw", xw, [128, 2, 2, 40], BF16, eng=nc.scalar)
        dtw_sb = load_w("dtw", dtw, [RK, 2, DI], BF16, eng=nc.scalar)
        dtb_sb = load_w("dtb", dtb, [128, 2, 2, 1], eng=nc.gpsimd)
        dpc_sb = load_w("dpc", dpc, [128, 2, 2, 1], eng=nc.gpsimd)
        wout_sb = load_w("wout", wout, [128, 2, C], BF16, eng=nc.gpsimd)
        lnw_sb = load_w("lnw", lnw, [1, 4, C], F32, eng=nc.scalar)
        lnw_bf = wp.tile([1, 4, C], BF16, tag="lnw_bf")
        nc.scalar.copy(lnw_bf[:], lnw_sb[:])
        g_row = [lnw_bf[:, 0, :], lnw_bf[:, 2, :]]
        b_row = [lnw_bf[:, 1, :], lnw_bf[:, 3, :]]
        ones_bf = wp.tile([C, 1], BF16, tag="ones_bf")
        nc.vector.memset(ones_bf[:], 1.0)
        eps_sb = wp.tile([C, 1], F32, tag="eps")
        nc.vector.memset(eps_sb[:], EPS)
        neg1_row = wp.tile([1, CBT], BF16, tag="neg1_row")
        nc.vector.memset(neg1_row[:], -1.0)

        def layernorm(ln_i, src_f32, src_bf, sq_bf, dst):
            """dst = (src - mean)/std * g + b over the partition dim.
            Folded: dst = src*(g*rstd)_r - (g*mean*rstd - b)_r."""
            ps_s = psC.tile([1, CBT], F32, tag="ps_stat", name="ps_s")
            ps_q = psC.tile([1, CBT], F32, tag="ps_stat", name="ps_q")
            nc.tensor.matmul(ps_s[:], ones_bf[:], src_bf, start=True, stop=True)
            nc.tensor.matmul(ps_q[:], ones_bf[:], sq_bf, start=True, stop=True)
            s_sb = stp.tile([1, CBT], F32, tag="s_sb")
            nc.scalar.copy(s_sb[:], ps_s[:])
            m2 = stp.tile([1, CBT], F32, tag="m2")
            nc.vector.tensor_mul(m2[:], s_sb[:], s_sb[:])
            varI = stp.tile([1, CBT], F32, tag="varI")
            # varI = C*ps_q - ps_s^2 = C^2 * var
            nc.vector.scalar_tensor_tensor(varI[:], ps_q[:], float(C), m2[:],
                                           ALU.mult, ALU.subtract)
            # rstd = exp(-0.5*ln(varI/C^2 + eps))
            nc.scalar.activation(varI[:], varI[:], AF.Ln,
                                 scale=1.0 / (C * C), bias=eps_sb[0:1, 0:1])
            nc.scalar.activation(varI[:], varI[:], AF.Exp, scale=-0.5)
            rstd_bf = stp.tile([1, CBT], BF16, tag="rstd_bf")
            nc.scalar.copy(rstd_bf[:], varI[:])
            # mr = mean*rstd = (ps_s/C)*rstd
            mr_bf = stp.tile([1, CBT], BF16, tag="mr_bf")
            nc.vector.scalar_tensor_tensor(mr_bf[:], s_sb[:], 1.0 / C,
                                           varI[:], ALU.mult, ALU.mult)
            grstd_r = psC.tile([C, CBT], F32, tag="ps_stat", name="grstd_r")
            nc.tensor.matmul(grstd_r[:], g_row[ln_i], rstd_bf[:],
                             start=True, stop=True)
            mgb_r = psC.tile([C, CBT], F32, tag="ps_stat", name="mgb_r")
            nc.tensor.matmul(mgb_r[:], b_row[ln_i], neg1_row[:],
                             start=True, stop=False)
            nc.tensor.matmul(mgb_r[:], g_row[ln_i], mr_bf[:],
                             start=False, stop=True)
            tln = sp.tile([C, CBT], F32, tag="ln_tmp")
            nc.vector.tensor_mul(tln[:], src_f32, grstd_r[:])
            nc.vector.tensor_sub(dst, tln[:], mgb_r[:])

        def front_head(ch):
            """u DMA + bf16 casts + LN1 stat matmuls (Act/PE only)."""
            b0 = ch * CB
            u = sp.tile([C, CB, T], F32, tag="u", bufs=3)
            nc.sync.dma_start(u[:], xin[:, b0:b0 + CB, :])
            uf = u[:].rearrange("p b t -> p (b t)")
            u_bf = sp.tile([C, CBT], BF16, tag="u_bf")
            nc.scalar.copy(u_bf[:], uf)
            sq_bf = sp.tile([C, CBT], BF16, tag="sq_bf")
            nc.scalar.square(sq_bf[:], uf)
            ps_s = psC.tile([1, CBT], F32, tag="ps_stat", name="ps_s")
            ps_q = psC.tile([1, CBT], F32, tag="ps_stat", name="ps_q")
            nc.tensor.matmul(ps_s[:], ones_bf[:], u_bf[:],
                             start=True, stop=True)
            nc.tensor.matmul(ps_q[:], ones_bf[:], sq_bf[:],
                             start=True, stop=True)
            s_sb = stp.tile([1, CBT], F32, tag="s_sb")
            nc.scalar.copy(s_sb[:], ps_s[:])
            return dict(ch=ch, uf=uf, ps_q=ps_q, s_sb=s_sb)

        def front_rest(hd):
            """LN1 stats (DVE) + apply + in_proj + conv(PE) + xproj +
            B/C broadcast + dtproj + dA(ti=0)."""
            ch, uf = hd["ch"], hd["uf"]
            ps_q, s_sb = hd["ps_q"], hd["s_sb"]
            m2 = stp.tile([1, CBT], F32, tag="m2")
            nc.vector.tensor_mul(m2[:], s_sb[:], s_sb[:])
            varI = stp.tile([1, CBT], F32, tag="varI")
            nc.vector.scalar_tensor_tensor(varI[:], ps_q[:], float(C), m2[:],
                                           ALU.mult, ALU.subtract)
            nc.scalar.activation(varI[:], varI[:], AF.Ln,
                                 scale=1.0 / (C * C), bias=eps_sb[0:1, 0:1])
            nc.scalar.activation(varI[:], varI[:], AF.Exp, scale=-0.5)
            rstd_bf = stp.tile([1, CBT], BF16, tag="rstd_bf")
            nc.scalar.copy(rstd_bf[:], varI[:])
            mr_bf = stp.tile([1, CBT], BF16, tag="mr_bf")
            nc.vector.scalar_tensor_tensor(mr_bf[:], s_sb[:], 1.0 / C,
                                           varI[:], ALU.mult, ALU.mult)
            grstd_r = psC.tile([C, CBT], F32, tag="ps_stat", name="grstd_r")
            nc.tensor.matmul(grstd_r[:], g_row[0], rstd_bf[:],
                             start=True, stop=True)
            mgb_r = psC.tile([C, CBT], F32, tag="ps_stat", name="mgb_r")
            nc.tensor.matmul(mgb_r[:], b_row[0], neg1_row[:],
                             start=True, stop=False)
            nc.tensor.matmul(mgb_r[:], g_row[0], mr_bf[:],
                             start=False, stop=True)
            hln = sp.tile([C, CBT], BF16, tag="hln")
            tln = sp.tile([C, CBT], F32, tag="ln_tmp")
            nc.vector.tensor_mul(tln[:], uf, grstd_r[:])
            nc.vector.tensor_sub(hln[:], tln[:], mgb_r[:])

            # in_proj (m-tiles: xx0 xx1 z0 z1)
            xx = [sp.tile([128, CB, T], BF16, tag=f"xx{ti}", name=f"xx{ti}")
                  for ti in range(2)]
            sz = [sp.tile([128, CB, T], BF16, tag=f"sz{ti}", name=f"sz{ti}")
                  for ti in range(2)]
            for mt in range(4):
                ps_xz = psA.tile([128, CBT], F32, tag="ps_xz")
                nc.tensor.matmul(ps_xz[:], w_in_sb[:, mt * 128:(mt + 1) * 128],
                                 hln[:], start=True, stop=True)
                dst = xx[mt][:] if mt < 2 else sz[mt - 2][:]
                pv = ps_xz[:].rearrange("p (b t) -> p b t", t=T)
                if mt < 2:
                    nc.scalar.copy(dst, pv)
                else:
                    nc.scalar.activation(dst, pv, AF.Silu)

            # depthwise conv on PE: shifted diagonal matmuls
            # xc2 combined [p, br, ti, b, t]
            xc2 = sp.tile([128, 2, 2, CB, T], BF16, tag="xc", name="xc2")
            for ti in range(2):
                xxv = xx[ti][:]
                for br in range(2):
                    ps_cv = psA.tile([128, CB, T], F32, tag="ps_xz",
                                     name=f"ps_cv{ti}{br}")
                    nc.tensor.matmul(ps_cv[:], convd_sb[:, br, ti, 3, :],
                                     xxv, start=True, stop=False)
                    for k in range(3):
                        s = 3 - k
                        if br == 0:
                            dst, src = ps_cv[:, :, s:], xxv[:, :, :T - s]
                        else:
                            dst, src = ps_cv[:, :, :T - s], xxv[:, :, s:]
                        nc.tensor.matmul(dst, convd_sb[:, br, ti, k, :], src,
                                         start=False, stop=(k == 2))
                    nc.scalar.activation(xc2[:, br, ti, :, :], ps_cv[:],
                                         AF.Silu, bias=convb_sb[:, br, ti, 0:1])

            # xproj -> x_dbl [40, CBT] per branch; dtproj chained per
            # branch right after its dtraw copy (shortest path to dt2);
            # each branch's B/C staging DMA issues as soon as its bc_bf
            # copy lands so the broadcast round trip starts earliest
            dtraw, bc_bf = [None, None], [None, None]
            dt2 = sp.tile([128, 2, 2, CB, T], BF16, tag="dt", name="dt2")
            b1d = drp.tile([2, DS, CB, T], BF16, tag="b1d")
            c1d = drp.tile([2, DS, CB, T], BF16, tag="c1d")
            for br in range(2):
                ps_xd = psA.tile([40, CBT], F32, tag="ps_m",
                                 name=f"ps_xd{br}")
                for ti in range(2):
                    nc.tensor.matmul(ps_xd[:], xw_sb[:, br, ti, :],
                                     xc2[:, br, ti, :, :].rearrange(
                                         "p b t -> p (b t)"),
                                     start=(ti == 0), stop=(ti == 1))
                dtraw[br] = stp.tile([RK, CBT], BF16, tag=f"dtraw{br}",
                                     name=f"dtraw{br}")
                nc.scalar.copy(dtraw[br][:], ps_xd[32:40, :])
                bc_bf[br] = stp.tile([32, CBT], BF16, tag=f"bcbf{br}",
                                     name=f"bcbf{br}")
                nc.scalar.copy(bc_bf[br][:], ps_xd[0:32, :])
                nc.sync.dma_start(b1d[br, :, :, :],
                                  bc_bf[br][0:DS, :].rearrange(
                                      "p (b t) -> p b t", t=T))
                nc.sync.dma_start(c1d[br, :, :, :],
                                  bc_bf[br][DS:32, :].rearrange(
                                      "p (b t) -> p b t", t=T))
                for ti in range(2):
                    ps_dt = psA.tile([128, CBT], F32, tag="ps_m",
                                     name=f"ps_dt{br}{ti}")
                    nc.tensor.matmul(ps_dt[:],
                                     dtw_sb[:, br, ti * 128:(ti + 1) * 128],
                                     dtraw[br][:], start=True, stop=True)
                    slab = dt2[:, br, ti, :, :]
                    nc.scalar.activation(
                        slab, ps_dt[:].rearrange("p (b t) -> p b t", t=T),
                        AF.Exp, bias=dtb_sb[:, br, ti, 0:1])
                    nc.scalar.activation(slab, slab, AF.Ln, bias=1.0)

            brep = bcp.tile([128, 2 * DS * CBT], BF16, tag="brep")
            nc.sync.dma_start(
                brep[:], _pbcast(b1d[:].rearrange("a n b t -> (a n b t)")))
            crep = bcp.tile([128, 2 * DS * CBT], BF16, tag="crep")
            nc.sync.dma_start(
                crep[:], _pbcast(c1d[:].rearrange("a n b t -> (a n b t)")))


            st = dict(ch=ch, uf=uf, xc2=xc2[:], sz=sz, dt2=dt2[:],
                      brep=brep, crep=crep, dA=None, dBu=None)
            prep_act(st)
            return st

        def prep_act(st):
            """dA (Act, both d-tiles per act) + segment resets (Pool).
            dA layout [p, br, ti, n, b, t]; the act dst slab for fixed n
            iterates (br, ti, b, t) matching dt2's element order."""
            dt2 = st["dt2"]
            dA = bp.tile([128, 2, 2, DS, CB, T], BF16, tag="dA",
                         bufs=1)
            for n in range(DS):
                nc.scalar.activation(dA[:, :, :, n, :, :], dt2[:],
                                     AF.Exp, scale=float(a_pow[n]))
            nc.gpsimd.memset(dA[:, 0, :, :, :, 0:1], 0.0)
            nc.gpsimd.memset(dA[:, 1, :, :, :, T - 1:T], 0.0)
            st["dA"] = dA

        def prep_mul(st):
            """du2 + dBu (DVE), one combined op each; emitted after back(c)
            so dtproj(c+1) has a full chunk of slack."""
            dt2, xc2 = st["dt2"], st["xc2"]
            brep6 = _zstride(st["brep"][:].rearrange(
                "p (a n b t) -> p a n b t", a=2, n=DS, b=CB), 1, 2)
            du2 = bp.tile([128, 2, 2, CB, T], BF16, tag="du2")
            nc.vector.tensor_mul(du2[:], dt2[:], xc2[:])
            dBu = bp.tile([128, 2, 2, DS, CB, T], BF16, tag="dBu",
                          bufs=1)
            nc.vector.tensor_mul(dBu[:], _zstride(du2[:], 2, DS), brep6)
            st["dBu"] = dBu

        def back(st):
            """Per-branch merged scan over (ti,n,b,t) + hc + n-reduce +
            gating + out_proj accumulate."""
            xc2, sz = st["xc2"], st["sz"]
            dA, dBu = st["dA"], st["dBu"]
            crep6 = _zstride(st["crep"][:].rearrange(
                "p (a n b t) -> p a n b t", a=2, n=DS, b=CB), 1, 2)
            st["ps_o"] = psB.tile([128, CBT], F32, tag="ps_o", name="ps_o")
            ps_o = st["ps_o"]
            flat = "p ti n b t -> p (ti n b t)"

            hb = [None, None]
            for br in range(2):
                h = bp.tile([128, 2, DS, CB, T], BF16, tag="h", bufs=1,
                            name=f"h{br}")
                hb[br] = h
                if br == 0:
                    nc.vector.tensor_tensor_scan(
                        h[:].rearrange(flat), dA[:, 0].rearrange(flat),
                        dBu[:, 0].rearrange(flat), 0.0, ALU.mult, ALU.add)
                else:
                    nc.vector.tensor_tensor_scan(
                        _rev_t(h[:].rearrange(flat)),
                        _rev_t(dA[:, 1].rearrange(flat)),
                        _rev_t(dBu[:, 1].rearrange(flat)),
                        0.0, ALU.mult, ALU.add)
                nc.vector.tensor_mul(h[:], h[:], crep6[:, br])
                for w in (8, 4, 2, 1):
                    nc.vector.tensor_add(h[:, :, 0:w, :, :],
                                         h[:, :, 0:w, :, :],
                                         h[:, :, w:2 * w, :, :])
                for ti in range(2):
                    y = sp.tile([128, CB, T], BF16, tag=f"y{br}{ti}",
                                name=f"y{br}{ti}")
                    nc.vector.scalar_tensor_tensor(
                        y[:], xc2[:, br, ti, :, :], dpc_sb[:, br, ti, 0:1],
                        h[:, ti, 0, :, :], ALU.mult, ALU.add)
                    st[f"y{br}{ti}"] = y
            for ti in range(2):
                ypre = sp.tile([128, CB, T], BF16, tag=f"ypre{ti}",
                               name=f"ypre{ti}")
                nc.vector.tensor_add(ypre[:], st[f"y0{ti}"][:],
                                     st[f"y1{ti}"][:])
                nc.vector.tensor_mul(ypre[:], ypre[:], sz[ti][:])
                nc.tensor.matmul(ps_o[:], wout_sb[:, ti, :],
                                 ypre[:].rearrange("p b t -> p (b t)"),
                                 start=(ti == 0), stop=(ti == 1))

        def tail(st):
            """LN2 + residual + store (runs one iteration later)."""
            ch, uf, ps_o = st["ch"], st["uf"], st["ps_o"]
            b0 = ch * CB
            o_sb = sp.tile([C, CBT], F32, tag="o_sb")
            nc.scalar.copy(o_sb[:], ps_o[:])
            o_bf = sp.tile([C, CBT], BF16, tag="o_bf")
            nc.scalar.copy(o_bf[:], ps_o[:])
            sq2_bf = sp.tile([C, CBT], BF16, tag="sq2_bf")
            nc.scalar.square(sq2_bf[:], ps_o[:])
            o_ln = sp.tile([C, CBT], F32, tag="o_ln")
            layernorm(1, o_sb[:], o_bf[:], sq2_bf[:], o_ln[:])
            nc.vector.tensor_add(o_ln[:], o_ln[:], uf)
            nc.sync.dma_start(out[:, b0:b0 + CB, :],
                              o_ln[:].rearrange("p (b t) -> p b t", t=T))

        # pipeline per iteration (steady state):
        #   tail(c-1) | front_head(c+1) | p1(c) | front_rest(c+1) | p2(c)
        #   | du2/dBu/dA(c+1)
        st = front_rest(front_head(0))
        prep_mul(st)
        done = None
        for ch in range(NCHUNK):
            nxt = front_rest(front_head(ch + 1)) \
                if ch + 1 < NCHUNK else None
            back(st)
            if done is not None:
                tail(done)
            if nxt is not None:
                prep_mul(nxt)
            done = st
            st = nxt
        tail(done)

    nc.finalize()
    return nc


def _prep(inputs):
    f = lambda k: np.ascontiguousarray(np.asarray(inputs[k], np.float32))
    bf = lambda a: np.ascontiguousarray(np.asarray(a, BF))
    x = f("x")
    u_all = x.transpose(0, 2, 1, 3).reshape(B * N, T, C)
    u_pad = np.zeros((BSEQ, T, C), np.float32)
    u_pad[:B * N] = u_all
    xin = [np.ascontiguousarray(u_pad[i * BC:(i + 1) * BC].transpose(2, 0, 1))
           for i in range(NCORES)]

    A = -np.exp(f("A_log"))
    Ab = -np.exp(f("A_b_log"))
    assert np.allclose(A, A[0:1], rtol=1e-5), "A must be d-independent"
    assert np.allclose(Ab, A, rtol=1e-5), "A_b must equal A"
    a_pow = [float(v) for v in A[0]]

    cw = np.stack([f("conv_w")[:, 0, :], f("conv_w_b")[:, 0, :]])   # [2,256,4]
    cb = np.stack([f("conv_b"), f("conv_b_b")])[..., None]          # [2,256,1]
    convd = np.zeros((128, 2, 2, 4, 128), np.float32)
    rng = np.arange(128)
    for br in range(2):
        for ti in range(2):
            for k in range(4):
                convd[rng, br, ti, k, rng] = cw[br, ti * 128:(ti + 1) * 128, k]
    xw_ro = np.concatenate([f("xproj_w")[RK:], f("xproj_w")[:RK]])
    xw_ro_b = np.concatenate([f("xproj_w_b")[RK:], f("xproj_w_b")[:RK]])
    xwm = np.stack([xw_ro, xw_ro_b]).transpose(0, 2, 1)
    dtwm = np.stack([f("dtproj_w"), f("dtproj_w_b")]).transpose(0, 2, 1)
    dtbm = np.stack([f("dtproj_b"), f("dtproj_b_b")])[..., None]
    shared = {
        "w_in": bf(f("in_proj_w").T),
        "convd": bf(convd),
        "convb": np.ascontiguousarray(
            cb.reshape(2, 2, 128, 1).transpose(2, 0, 1, 3)),
        "xw": bf(xwm.reshape(2, 2, 128, 40).transpose(2, 0, 1, 3)),
        "dtw": bf(dtwm.transpose(1, 0, 2)),                         # [8,2,256]
        "dtb": np.ascontiguousarray(
            dtbm.reshape(2, 2, 128, 1).transpose(2, 0, 1, 3)),
        "dpc": np.ascontiguousarray(
            np.stack([f("Dp"), f("Dp_b")])[..., None]
            .reshape(2, 2, 128, 1).transpose(2, 0, 1, 3)),
        "wout": bf(f("out_proj_w").T.reshape(2, 128, 128).transpose(1, 0, 2)),
        "lnw": np.ascontiguousarray(
            np.stack([f("ln1_g"), f("ln1_b"), f("ln2_g"), f("ln2_b")])
            .reshape(1, 4, C)),
    }
    return xin, shared, a_pow


def _unshard(core_outs):
    y = np.stack(core_outs)                       # [8, C, BC, T]
    y = y.transpose(0, 2, 3, 1).reshape(BSEQ, T, C)[:B * N]
    return np.ascontiguousarray(
        y.reshape(B, N, T, C).transpose(0, 2, 1, 3))


_CACHE = {}


def kernel(_trace=False, **inputs):
    xin, shared, a_pow = _prep(inputs)
    if "prog" not in _CACHE:
        _CACHE["prog"] = build_program(a_pow)
    nc = _CACHE["prog"]
    in_maps = [dict(shared, xin=xin[i]) for i in range(NCORES)]
    res = run_bass_kernel_spmd(nc, in_maps, core_ids=list(range(NCORES)),
                               trace=_trace)
    out = _unshard([r["out"] for r in res.results])
    if _trace:
        kernel.last_results = res
    return out
